# revision 10
# baseline (speedup 1.0000x reference)
"""Permutation-invariant SDR loss kernel for Trainium2 (8 NeuronCores).

Full inputs: pr_batch [32, 4, 131072] f32, t_batch [32, 4, 131072] f32.
Output: scalar f32 loss = -mean_b max_perm mean_j 10*log10((Td_j+eps)/(e_dot+eps)).

Sharding: pure data parallel on batch — 4 examples per core. Each core
computes, per example, the 24 length-T dot products (Pd_i, Td_j, C_ij),
then the tiny permutation/log/max tail on-device, outputting one value
per example. Host gathers 32 values, scales, means, negates.

Math used on device (exactly equivalent to the reference):
  mean_snr[p] = (10/(4 ln10)) * (sum_j ln(Td_j+eps) - sum_j ln(e_dot[p,j]+eps))
  best[e]     = sum_j ln(Td_j+eps) - min_p sum_j ln(e_dot[p,j]+eps)
Host: loss = -mean(best * 10/(4 ln10)).
"""

import itertools
import os

import numpy as np

import concourse.bacc as bacc
import concourse.bass as bass
import concourse.mybir as mybir
import concourse.tile as tile
from concourse.bass_utils import run_bass_kernel_spmd

N_CORES = 8
B, NSRC, T = 32, 4, 131072
EB = B // N_CORES  # examples per core
P = 128
F = T // P  # 1024 free elems per partition per source
EPS = 1e-9
NPERM = 24
NV = 2 * NSRC + NSRC * NSRC  # 24 dot products per example
NE = NPERM * NSRC + NSRC  # 100 output rows of the selector matmul
SNR_SCALE = 10.0 / (NSRC * np.log(10.0))

AF = mybir.ActivationFunctionType
ALU = mybir.AluOpType
FP32 = mybir.dt.float32


NVPAD = 64  # v_sb partition layout: 0..3 Pd_i, 4..7 Td_j, 32..47 C[i,j], rest 0


def _selector_matrix() -> np.ndarray:
    """M [NE, NVPAD] mapping the dot-vector v to [e_dot rows (96), Td rows (4)].

    v partition layout: k in 0..3 -> Pd_i, 4..7 -> Td_j, 32+i*4+j -> C[i,j]
    (C lives at partition base 32 because engine partition bases must be
    32-aligned). e_dot[(p,j)] = Pd[perm[p][j]] - 2*C[perm[p][j], j] + Td[j].
    """
    perms = list(itertools.permutations(range(NSRC)))
    M = np.zeros((NE, NVPAD), dtype=np.float32)
    for pi, perm in enumerate(perms):
        for j in range(NSRC):
            m = pi * NSRC + j
            M[m, perm[j]] += 1.0
            M[m, NSRC + j] += 1.0
            M[m, 32 + perm[j] * NSRC + j] += -2.0
    for j in range(NSRC):
        M[NPERM * NSRC + j, NSRC + j] = 1.0
    return M


def _build_bass() -> bass.Bass:
    # Bacc (not plain Bass): its generate_event_semaphores pass splits sync
    # waits to <=1 per instruction, which this walrus build requires.
    nc = bacc.Bacc("TRN2")
    pr_h = nc.dram_tensor("pr", [EB, NSRC, T], FP32, kind="ExternalInput")
    t_h = nc.dram_tensor("t", [EB, NSRC, T], FP32, kind="ExternalInput")
    out_h = nc.dram_tensor("best", [EB, 1], FP32, kind="ExternalOutput")

    m2t_h = nc.inline_tensor(np.ascontiguousarray(_selector_matrix().T))  # [NVPAD, NE]
    ones_h = nc.inline_tensor(np.ones((P, 1), np.float32))

    with tile.TileContext(nc) as tc:
        with (
            tc.tile_pool(name="data", bufs=2) as data,
            tc.tile_pool(name="scratch", bufs=2) as scratch,
            tc.tile_pool(name="accum", bufs=2) as accum,
            tc.tile_pool(name="small", bufs=1) as small,
            tc.tile_pool(name="psum", bufs=1, space="PSUM") as psum,
        ):
            ones_sb = small.tile([P, 1], FP32)
            nc.sync.dma_start(out=ones_sb, in_=ones_h[:, :])
            m2t_sb = small.tile([NVPAD, NE], FP32)
            nc.sync.dma_start(out=m2t_sb, in_=m2t_h[:, :])

            # Per-example partition-sums of the 24 dots land here (PE reduce).
            psum_va = psum.tile([2 * NSRC, EB], FP32, tag="va")
            psum_vd = psum.tile([NSRC * NSRC, EB], FP32, tag="vd")

            for e in range(EB):
                pr_t = data.tile([P, NSRC, F], FP32, tag="pr")
                t_t = data.tile([P, NSRC, F], FP32, tag="t")
                nc.sync.dma_start(
                    out=pr_t, in_=pr_h[e].rearrange("s (p f) -> p s f", p=P)
                )
                nc.sync.dma_start(
                    out=t_t, in_=t_h[e].rearrange("s (p f) -> p s f", p=P)
                )

                a_act = accum.tile([P, 2 * NSRC], FP32, tag="a_act")
                a_dve = accum.tile([P, NSRC * NSRC], FP32, tag="a_dve")
                sc_a = scratch.tile([P, F], FP32, tag="sa")
                sc_v = scratch.tile([P, F], FP32, tag="sv")

                # Self dots on ACT: square with free-dim accumulate.
                for i in range(NSRC):
                    nc.scalar.activation(
                        out=sc_a,
                        in_=pr_t[:, i, :],
                        func=AF.Square,
                        accum_out=a_act[:, i : i + 1],
                    )
                for j in range(NSRC):
                    nc.scalar.activation(
                        out=sc_a,
                        in_=t_t[:, j, :],
                        func=AF.Square,
                        accum_out=a_act[:, NSRC + j : NSRC + j + 1],
                    )
                # Cross dots on DVE: fused multiply + free-dim accumulate.
                for i in range(NSRC):
                    for j in range(NSRC):
                        k = i * NSRC + j
                        nc.vector.scalar_tensor_tensor(
                            out=sc_v,
                            in0=pr_t[:, i, :],
                            scalar=1.0,
                            in1=t_t[:, j, :],
                            op0=ALU.mult,
                            op1=ALU.mult,
                            accum_out=a_dve[:, k : k + 1],
                        )
                # Partition-reduce the 24 per-partition sums via ones-matmul.
                nc.tensor.matmul(
                    psum_va[:, e : e + 1], lhsT=a_act, rhs=ones_sb, start=True, stop=True
                )
                nc.tensor.matmul(
                    psum_vd[:, e : e + 1], lhsT=a_dve, rhs=ones_sb, start=True, stop=True
                )

            # Tail: tiny. v_sb [NVPAD, EB] -> selector matmul -> ln -> reduces.
            v_sb = small.tile([NVPAD, EB], FP32)
            nc.vector.memset(v_sb, 0.0)
            nc.scalar.copy(out=v_sb[0 : 2 * NSRC, :], in_=psum_va[:, :])
            nc.scalar.copy(out=v_sb[32 : 32 + NSRC * NSRC, :], in_=psum_vd[:, :])

            psum_e = psum.tile([EB, NE], FP32, tag="ed")
            nc.tensor.matmul(psum_e, lhsT=v_sb, rhs=m2t_sb, start=True, stop=True)

            eps_sb = small.tile([EB, 1], FP32)
            nc.vector.memset(eps_sb, float(EPS))
            ln_sb = small.tile([EB, NE], FP32)
            nc.scalar.activation(
                out=ln_sb, in_=psum_e, func=AF.Ln, bias=eps_sb[:, :], scale=1.0
            )
            s_perm = small.tile([EB, NPERM], FP32)
            nc.vector.tensor_reduce(
                out=s_perm,
                in_=ln_sb[:, 0 : NPERM * NSRC].rearrange("p (q j) -> p q j", j=NSRC),
                axis=mybir.AxisListType.X,
                op=ALU.add,
            )
            tsum = small.tile([EB, 1], FP32)
            nc.vector.tensor_reduce(
                out=tsum,
                in_=ln_sb[:, NPERM * NSRC : NE],
                axis=mybir.AxisListType.X,
                op=ALU.add,
            )
            mmin = small.tile([EB, 1], FP32)
            nc.vector.tensor_reduce(
                out=mmin, in_=s_perm, axis=mybir.AxisListType.X, op=ALU.min
            )
            diff = small.tile([EB, 1], FP32)
            nc.vector.tensor_sub(diff, tsum, mmin)
            nc.sync.dma_start(out=out_h[:, :], in_=diff)

    nc.finalize()
    return nc


_NC_CACHE = None


def _get_nc() -> bass.Bass:
    global _NC_CACHE
    if _NC_CACHE is None:
        _NC_CACHE = _build_bass()
    return _NC_CACHE


def run_device(pr_batch: np.ndarray, t_batch: np.ndarray, **run_kwargs):
    pr = np.ascontiguousarray(pr_batch, dtype=np.float32)
    t = np.ascontiguousarray(t_batch, dtype=np.float32)
    assert pr.shape == (B, NSRC, T) and t.shape == (B, NSRC, T)
    nc = _get_nc()
    in_maps = [
        {"pr": pr[c * EB : (c + 1) * EB], "t": t[c * EB : (c + 1) * EB]}
        for c in range(N_CORES)
    ]
    return run_bass_kernel_spmd(nc, in_maps, core_ids=list(range(N_CORES)), **run_kwargs)


def kernel(pr_batch: np.ndarray, t_batch: np.ndarray) -> np.ndarray:
    res = run_device(pr_batch, t_batch)
    best = np.concatenate([r["best"].reshape(-1) for r in res.results])
    loss = -np.float32(SNR_SCALE) * best.astype(np.float32).mean(dtype=np.float32)
    return np.float32(loss)


if __name__ == "__main__":
    rng = np.random.default_rng(0)
    pr = rng.standard_normal((B, NSRC, T), dtype=np.float32)
    t = rng.standard_normal((B, NSRC, T), dtype=np.float32)
    print(kernel(pr, t))


# revision 11
# speedup vs baseline: 1.0957x; 1.0957x over previous
"""Permutation-invariant SDR loss kernel for Trainium2 (8 NeuronCores).

Full inputs: pr_batch [32, 4, 131072] f32, t_batch [32, 4, 131072] f32.
Output: scalar f32 loss = -mean_b max_perm mean_j 10*log10((Td_j+eps)/(e_dot+eps)).

Sharding: pure data parallel on batch — 4 examples per core. Each core
computes, per example, the 24 length-T dot products (Pd_i, Td_j, C_ij),
then the tiny permutation/log/max tail on-device, outputting one value
per example. Host gathers 32 values, scales, means, negates.

Perf design (memory-bound target):
- SWDGE DMA casts f32->bf16 during the HBM->SBUF load (no on-chip
  conversion passes); HBM read volume is the roofline (~16.8 MB/core).
- Each example is split into two half-T chunks so compute starts after
  ~2 MB instead of ~4 MB.
- Cross dots: DVE scalar_tensor_tensor (mult+mult) with free-dim
  accumulator, bf16 2x_1P mode.
- Self dots: ACT Square activation with free-dim accumulator.
- Partition reduction of all 48 per-half accumulator columns: two
  ones-matmuls per example on the (idle) TensorEngine.
- Tail (selector matmul, ln, reduces) is tiny and exactly equivalent to
  the reference math:
    best[e] = sum_j ln(Td_j+eps) - min_p sum_j ln(e_dot[p,j]+eps)
  Host: loss = -mean(best * 10/(4 ln10)).
"""

import itertools
import os

import numpy as np

import concourse.bacc as bacc
import concourse.bass as bass
import concourse.mybir as mybir
import concourse.tile as tile
from concourse.bass_utils import run_bass_kernel_spmd

N_CORES = 8
B, NSRC, T = 32, 4, 131072
EB = B // N_CORES  # examples per core
P = 128
F = T // P  # 1024 free elems per partition per source
HF = F // 2  # half-chunk free size
EPS = 1e-9
NPERM = 24
NE = NPERM * NSRC + NSRC  # 100 output rows of the selector matmul
NVPAD = 64  # v_sb partitions: 0..15 = Pd/Td per half, 32..63 = C per half
SNR_SCALE = 10.0 / (NSRC * np.log(10.0))

AF = mybir.ActivationFunctionType
ALU = mybir.AluOpType
FP32 = mybir.dt.float32
BF16 = mybir.dt.bfloat16


def _selector_matrix() -> np.ndarray:
    """M [NE, NVPAD] mapping the per-half dot-vector v to
    [e_dot rows (96), Td rows (4)].

    v partition layout (half h in {0,1}):
      h*8 + i       -> Pd_i (half h)
      h*8 + 4 + j   -> Td_j (half h)
      32 + h*16 + i*4 + j -> C[i,j] (half h)
    Summing halves is folded into the matmul coefficients.
    e_dot[(p,j)] = Pd[perm[p][j]] - 2*C[perm[p][j], j] + Td[j].
    """
    perms = list(itertools.permutations(range(NSRC)))
    M = np.zeros((NE, NVPAD), dtype=np.float32)
    for pi, perm in enumerate(perms):
        for j in range(NSRC):
            m = pi * NSRC + j
            for h in range(2):
                M[m, h * 8 + perm[j]] += 1.0
                M[m, h * 8 + 4 + j] += 1.0
                M[m, 32 + h * 16 + perm[j] * NSRC + j] += -2.0
    for j in range(NSRC):
        for h in range(2):
            M[NPERM * NSRC + j, h * 8 + 4 + j] = 1.0
    return M


def _build_bass() -> bass.Bass:
    # Bacc (not plain Bass): its generate_event_semaphores pass splits sync
    # waits to <=1 per instruction, which this walrus build requires.
    nc = bacc.Bacc("TRN2")
    pr_h = nc.dram_tensor("pr", [EB, NSRC, T], FP32, kind="ExternalInput")
    t_h = nc.dram_tensor("t", [EB, NSRC, T], FP32, kind="ExternalInput")
    out_h = nc.dram_tensor("best", [EB, 1], FP32, kind="ExternalOutput")

    m2t_h = nc.inline_tensor(np.ascontiguousarray(_selector_matrix().T))  # [NVPAD, NE]
    ones_h = nc.inline_tensor(np.ones((P, 1), np.float32))

    with tile.TileContext(nc) as tc:
        with (
            tc.tile_pool(name="data", bufs=3) as data,
            tc.tile_pool(name="scratch", bufs=3) as scratch,
            tc.tile_pool(name="accum", bufs=2) as accum,
            tc.tile_pool(name="small", bufs=1) as small,
            tc.tile_pool(name="psum", bufs=1, space="PSUM") as psum,
        ):
            ones_sb = small.tile([P, 1], FP32)
            nc.sync.dma_start(out=ones_sb, in_=ones_h[:, :])
            m2t_sb = small.tile([NVPAD, NE], FP32)
            nc.sync.dma_start(out=m2t_sb, in_=m2t_h[:, :])

            psum_va = psum.tile([16, EB], FP32, tag="va")
            psum_vd = psum.tile([32, EB], FP32, tag="vd")

            for e in range(EB):
                a_act = accum.tile([P, 16], FP32, tag="a_act")
                a_dve = accum.tile([P, 32], FP32, tag="a_dve")
                for h in range(2):
                    pr_t = data.tile([P, NSRC, HF], BF16, tag="pr")
                    t_t = data.tile([P, NSRC, HF], BF16, tag="t")
                    src_pr = pr_h[e].rearrange("s (p f) -> p s f", p=P)
                    src_t = t_h[e].rearrange("s (p f) -> p s f", p=P)
                    # SWDGE DMA with f32 -> bf16 cast during transfer.
                    nc.gpsimd.dma_start(
                        out=pr_t, in_=src_pr[:, :, h * HF : (h + 1) * HF]
                    )
                    nc.gpsimd.dma_start(
                        out=t_t, in_=src_t[:, :, h * HF : (h + 1) * HF]
                    )

                    # Self dots on ACT: square with free-dim accumulate.
                    for i in range(NSRC):
                        sc_a = scratch.tile([P, HF], BF16, tag="sa")
                        nc.scalar.activation(
                            out=sc_a,
                            in_=pr_t[:, i, :],
                            func=AF.Square,
                            accum_out=a_act[:, h * 8 + i : h * 8 + i + 1],
                        )
                    for j in range(NSRC):
                        sc_a = scratch.tile([P, HF], BF16, tag="sa")
                        nc.scalar.activation(
                            out=sc_a,
                            in_=t_t[:, j, :],
                            func=AF.Square,
                            accum_out=a_act[:, h * 8 + 4 + j : h * 8 + 5 + j],
                        )
                    # Cross dots on DVE: fused multiply + free-dim accumulate.
                    for i in range(NSRC):
                        for j in range(NSRC):
                            k = h * 16 + i * NSRC + j
                            sc_v = scratch.tile([P, HF], BF16, tag="sv")
                            nc.vector.scalar_tensor_tensor(
                                out=sc_v,
                                in0=pr_t[:, i, :],
                                scalar=1.0,
                                in1=t_t[:, j, :],
                                op0=ALU.mult,
                                op1=ALU.mult,
                                accum_out=a_dve[:, k : k + 1],
                            )
                # Partition-reduce the accumulator columns via ones-matmul.
                nc.tensor.matmul(
                    psum_va[:, e : e + 1], lhsT=a_act, rhs=ones_sb, start=True, stop=True
                )
                nc.tensor.matmul(
                    psum_vd[:, e : e + 1], lhsT=a_dve, rhs=ones_sb, start=True, stop=True
                )

            # Tail: tiny. v_sb [NVPAD, EB] -> selector matmul -> ln -> reduces.
            v_sb = small.tile([NVPAD, EB], FP32)
            nc.vector.memset(v_sb, 0.0)
            nc.scalar.copy(out=v_sb[0:16, :], in_=psum_va[:, :])
            nc.scalar.copy(out=v_sb[32:64, :], in_=psum_vd[:, :])

            psum_e = psum.tile([EB, NE], FP32, tag="ed")
            nc.tensor.matmul(psum_e, lhsT=v_sb, rhs=m2t_sb, start=True, stop=True)

            eps_sb = small.tile([EB, 1], FP32)
            nc.vector.memset(eps_sb, float(EPS))
            ln_sb = small.tile([EB, NE], FP32)
            nc.scalar.activation(
                out=ln_sb, in_=psum_e, func=AF.Ln, bias=eps_sb[:, :], scale=1.0
            )
            s_perm = small.tile([EB, NPERM], FP32)
            nc.vector.tensor_reduce(
                out=s_perm,
                in_=ln_sb[:, 0 : NPERM * NSRC].rearrange("p (q j) -> p q j", j=NSRC),
                axis=mybir.AxisListType.X,
                op=ALU.add,
            )
            tsum = small.tile([EB, 1], FP32)
            nc.vector.tensor_reduce(
                out=tsum,
                in_=ln_sb[:, NPERM * NSRC : NE],
                axis=mybir.AxisListType.X,
                op=ALU.add,
            )
            mmin = small.tile([EB, 1], FP32)
            nc.vector.tensor_reduce(
                out=mmin, in_=s_perm, axis=mybir.AxisListType.X, op=ALU.min
            )
            diff = small.tile([EB, 1], FP32)
            nc.vector.tensor_sub(diff, tsum, mmin)
            nc.sync.dma_start(out=out_h[:, :], in_=diff)

    nc.finalize()
    return nc


_NC_CACHE = None


def _get_nc() -> bass.Bass:
    global _NC_CACHE
    if _NC_CACHE is None:
        _NC_CACHE = _build_bass()
    return _NC_CACHE


def run_device(pr_batch: np.ndarray, t_batch: np.ndarray, **run_kwargs):
    pr = np.ascontiguousarray(pr_batch, dtype=np.float32)
    t = np.ascontiguousarray(t_batch, dtype=np.float32)
    assert pr.shape == (B, NSRC, T) and t.shape == (B, NSRC, T)
    nc = _get_nc()
    in_maps = [
        {"pr": pr[c * EB : (c + 1) * EB], "t": t[c * EB : (c + 1) * EB]}
        for c in range(N_CORES)
    ]
    return run_bass_kernel_spmd(nc, in_maps, core_ids=list(range(N_CORES)), **run_kwargs)


def kernel(pr_batch: np.ndarray, t_batch: np.ndarray) -> np.ndarray:
    res = run_device(pr_batch, t_batch)
    best = np.concatenate([r["best"].reshape(-1) for r in res.results])
    loss = -np.float32(SNR_SCALE) * best.astype(np.float32).mean(dtype=np.float32)
    return np.float32(loss)


if __name__ == "__main__":
    rng = np.random.default_rng(0)
    pr = rng.standard_normal((B, NSRC, T), dtype=np.float32)
    t = rng.standard_normal((B, NSRC, T), dtype=np.float32)
    print(kernel(pr, t))


# revision 13
# speedup vs baseline: 1.5748x; 1.4372x over previous
"""Permutation-invariant SDR loss kernel for Trainium2 (8 NeuronCores).

Full inputs: pr_batch [32, 4, 131072] f32, t_batch [32, 4, 131072] f32.
Output: scalar f32 loss = -mean_b max_perm mean_j 10*log10((Td_j+eps)/(e_dot+eps)).

Sharding: pure data parallel on batch — 4 examples per core. Each core
computes, per example, the 24 length-T dot products (Pd_i, Td_j, C_ij),
then the tiny permutation/log/max tail on-device, outputting one value
per example. Host gathers 32 values, scales, means, negates.

Perf design (memory-bound target; HBM read ~16.8 MB/core is the floor):
- SWDGE DMA casts f32->bf16 during the HBM->SBUF load.
- Cross products pr_i*t_j: DVE tensor_tensor mult in bf16 (2x_1P mode,
  ~620ns per [128,1024]); the reduction of each product runs on the
  otherwise-idle TensorEngine as a ones-stationary matmul whose output
  AP folds the 512-column stream onto 16 PSUM columns (PSUM has_written
  accumulate; verified exact). One DVE reduce per example collapses the
  [128, 16 pairs x 16] PSUM block to [128, 16] per-partition partials.
- Self dots: ACT Square activation with free-dim accumulator.
- Partition reduction of accumulator columns: ones-matmul per example.
- Tail (selector matmul, ln, reduces) is exactly equivalent to:
    best[e] = sum_j ln(Td_j+eps) - min_p sum_j ln(e_dot[p,j]+eps)
  Host: loss = -mean(best * 10/(4 ln10)).
"""

import itertools
import os

import numpy as np

import concourse.bacc as bacc
import concourse.bass as bass
import concourse.mybir as mybir
import concourse.tile as tile
from concourse.bass_utils import run_bass_kernel_spmd

N_CORES = 8
B, NSRC, T = 32, 4, 131072
EB = B // N_CORES  # examples per core
P = 128
F = T // P  # 1024 free elems per partition per source
EPS = 1e-9
NPERM = 24
NPAIR = NSRC * NSRC  # 16 cross pairs
FOLD = 16  # PSUM columns per pair (stream folds 512 -> 16; >=16 avoids RAW)
NE = NPERM * NSRC + NSRC  # 100 output rows of the selector matmul
NVPAD = 64  # v_sb partitions: 0..7 = Pd/Td, 32..47 = C
SNR_SCALE = 10.0 / (NSRC * np.log(10.0))

AF = mybir.ActivationFunctionType
ALU = mybir.AluOpType
FP32 = mybir.dt.float32
BF16 = mybir.dt.bfloat16


def _selector_matrix() -> np.ndarray:
    """M [NE, NVPAD]: v rows 0..3 Pd_i, 4..7 Td_j, 32+i*4+j C[i,j].
    e_dot[(p,j)] = Pd[perm[p][j]] - 2*C[perm[p][j], j] + Td[j]; rows 96..99
    emit Td_j for the numerator."""
    perms = list(itertools.permutations(range(NSRC)))
    M = np.zeros((NE, NVPAD), dtype=np.float32)
    for pi, perm in enumerate(perms):
        for j in range(NSRC):
            m = pi * NSRC + j
            M[m, perm[j]] += 1.0
            M[m, 4 + j] += 1.0
            # C arrives replicated on all 128 partitions (PE colsum), so the
            # partition-sum in stage-A over-counts by 128x — fold 1/128 here.
            M[m, 32 + perm[j] * NSRC + j] += -2.0 / 128.0
    for j in range(NSRC):
        M[NPERM * NSRC + j, 4 + j] = 1.0
    return M


def _fold_ap(psum_tile, pair: int):
    """Matmul out AP: stream column n -> psum column pair*FOLD + (n % FOLD)."""
    sl = psum_tile[:, pair * FOLD : (pair + 1) * FOLD]
    return bass.AP(
        tensor=sl.tensor,
        offset=sl.offset,
        ap=[list(sl.ap[0]), [0, 512 // FOLD], [1, FOLD]],
    )


def _build_bass() -> bass.Bass:
    # Bacc (not plain Bass): generate_event_semaphores splits sync waits to
    # <=1 per instruction, which this walrus build requires.
    nc = bacc.Bacc("TRN2")
    pr_h = nc.dram_tensor("pr", [EB, NSRC, T], FP32, kind="ExternalInput")
    t_h = nc.dram_tensor("t", [EB, NSRC, T], FP32, kind="ExternalInput")
    out_h = nc.dram_tensor("best", [EB, 1], FP32, kind="ExternalOutput")

    m2t_h = nc.inline_tensor(np.ascontiguousarray(_selector_matrix().T))  # [NVPAD, NE]
    ones_h = nc.inline_tensor(np.ones((P, 1), np.float32))
    onesm_h = nc.inline_tensor(np.ones((P, P), np.float32))

    with tile.TileContext(nc) as tc:
        with (
            tc.tile_pool(name="data", bufs=3) as data,
            tc.tile_pool(name="prod", bufs=6) as prod,
            tc.tile_pool(name="scratch", bufs=2) as scratch,
            tc.tile_pool(name="accum", bufs=2) as accum,
            tc.tile_pool(name="small", bufs=1) as small,
            tc.tile_pool(name="psum", bufs=1, space="PSUM") as psum,
            tc.tile_pool(name="psumc", bufs=2, space="PSUM") as psumc,
        ):
            ones_sb = small.tile([P, 1], FP32)
            nc.sync.dma_start(out=ones_sb, in_=ones_h[:, :])
            onesm_f = small.tile([P, P], FP32)
            nc.sync.dma_start(out=onesm_f, in_=onesm_h[:, :])
            onesm = small.tile([P, P], BF16)
            nc.vector.tensor_copy(onesm, onesm_f)
            m2t_sb = small.tile([NVPAD, NE], FP32)
            nc.sync.dma_start(out=m2t_sb, in_=m2t_h[:, :])

            psum_va = psum.tile([8, EB], FP32, tag="va")
            psum_vd = psum.tile([NPAIR, EB], FP32, tag="vd")

            for e in range(EB):
                pr_t = data.tile([P, NSRC, F], BF16, tag="pr")
                t_t = data.tile([P, NSRC, F], BF16, tag="t")
                # SWDGE DMA with f32 -> bf16 cast during transfer.
                nc.gpsimd.dma_start(
                    out=pr_t, in_=pr_h[e].rearrange("s (p f) -> p s f", p=P)
                )
                nc.gpsimd.dma_start(
                    out=t_t, in_=t_h[e].rearrange("s (p f) -> p s f", p=P)
                )

                a_act = accum.tile([P, 8], FP32, tag="a_act")
                a_dve = accum.tile([P, NPAIR], FP32, tag="a_dve")

                # Self dots on ACT: square with free-dim accumulate.
                for i in range(NSRC):
                    sc_a = scratch.tile([P, F], BF16, tag="sa")
                    nc.scalar.activation(
                        out=sc_a,
                        in_=pr_t[:, i, :],
                        func=AF.Square,
                        accum_out=a_act[:, i : i + 1],
                    )
                for j in range(NSRC):
                    sc_a = scratch.tile([P, F], BF16, tag="sa")
                    nc.scalar.activation(
                        out=sc_a,
                        in_=t_t[:, j, :],
                        func=AF.Square,
                        accum_out=a_act[:, 4 + j : 5 + j],
                    )
                # Cross products on DVE (bf16 2x), reduced on PE via folded
                # ones-matmuls into psum_c, then one DVE reduce -> a_dve.
                psum_c = psumc.tile([P, NPAIR * FOLD], FP32, tag="pc")
                for i in range(NSRC):
                    for j in range(NSRC):
                        k = i * NSRC + j
                        z = prod.tile([P, F], BF16, tag="z")
                        nc.vector.tensor_mul(z[:, :], pr_t[:, i, :], t_t[:, j, :])
                        nc.tensor.matmul(
                            _fold_ap(psum_c, k),
                            lhsT=onesm,
                            rhs=z[:, 0:512],
                            start=True,
                            stop=False,
                        )
                        nc.tensor.matmul(
                            _fold_ap(psum_c, k),
                            lhsT=onesm,
                            rhs=z[:, 512:1024],
                            start=False,
                            stop=True,
                        )
                nc.vector.tensor_reduce(
                    out=a_dve,
                    in_=psum_c.rearrange("p (k f) -> p k f", f=FOLD),
                    axis=mybir.AxisListType.X,
                    op=ALU.add,
                )
                # Partition-reduce accumulator columns via ones-matmul.
                nc.tensor.matmul(
                    psum_va[:, e : e + 1], lhsT=a_act, rhs=ones_sb, start=True, stop=True
                )
                nc.tensor.matmul(
                    psum_vd[:, e : e + 1], lhsT=a_dve, rhs=ones_sb, start=True, stop=True
                )

            # Tail: tiny. v_sb [NVPAD, EB] -> selector matmul -> ln -> reduces.
            v_sb = small.tile([NVPAD, EB], FP32)
            nc.vector.memset(v_sb, 0.0)
            nc.scalar.copy(out=v_sb[0:8, :], in_=psum_va[:, :])
            nc.scalar.copy(out=v_sb[32 : 32 + NPAIR, :], in_=psum_vd[:, :])

            psum_e = psum.tile([EB, NE], FP32, tag="ed")
            nc.tensor.matmul(psum_e, lhsT=v_sb, rhs=m2t_sb, start=True, stop=True)

            eps_sb = small.tile([EB, 1], FP32)
            nc.vector.memset(eps_sb, float(EPS))
            ln_sb = small.tile([EB, NE], FP32)
            nc.scalar.activation(
                out=ln_sb, in_=psum_e, func=AF.Ln, bias=eps_sb[:, :], scale=1.0
            )
            s_perm = small.tile([EB, NPERM], FP32)
            nc.vector.tensor_reduce(
                out=s_perm,
                in_=ln_sb[:, 0 : NPERM * NSRC].rearrange("p (q j) -> p q j", j=NSRC),
                axis=mybir.AxisListType.X,
                op=ALU.add,
            )
            tsum = small.tile([EB, 1], FP32)
            nc.vector.tensor_reduce(
                out=tsum,
                in_=ln_sb[:, NPERM * NSRC : NE],
                axis=mybir.AxisListType.X,
                op=ALU.add,
            )
            mmin = small.tile([EB, 1], FP32)
            nc.vector.tensor_reduce(
                out=mmin, in_=s_perm, axis=mybir.AxisListType.X, op=ALU.min
            )
            diff = small.tile([EB, 1], FP32)
            nc.vector.tensor_sub(diff, tsum, mmin)
            nc.sync.dma_start(out=out_h[:, :], in_=diff)

    nc.finalize()
    return nc


_NC_CACHE = None


def _get_nc() -> bass.Bass:
    global _NC_CACHE
    if _NC_CACHE is None:
        _NC_CACHE = _build_bass()
    return _NC_CACHE


def run_device(pr_batch: np.ndarray, t_batch: np.ndarray, **run_kwargs):
    pr = np.ascontiguousarray(pr_batch, dtype=np.float32)
    t = np.ascontiguousarray(t_batch, dtype=np.float32)
    assert pr.shape == (B, NSRC, T) and t.shape == (B, NSRC, T)
    nc = _get_nc()
    in_maps = [
        {"pr": pr[c * EB : (c + 1) * EB], "t": t[c * EB : (c + 1) * EB]}
        for c in range(N_CORES)
    ]
    return run_bass_kernel_spmd(nc, in_maps, core_ids=list(range(N_CORES)), **run_kwargs)


def kernel(pr_batch: np.ndarray, t_batch: np.ndarray) -> np.ndarray:
    res = run_device(pr_batch, t_batch)
    best = np.concatenate([r["best"].reshape(-1) for r in res.results])
    loss = -np.float32(SNR_SCALE) * best.astype(np.float32).mean(dtype=np.float32)
    return np.float32(loss)


if __name__ == "__main__":
    rng = np.random.default_rng(0)
    pr = rng.standard_normal((B, NSRC, T), dtype=np.float32)
    t = rng.standard_normal((B, NSRC, T), dtype=np.float32)
    print(kernel(pr, t))


# revision 16
# speedup vs baseline: 1.6155x; 1.0259x over previous
"""Permutation-invariant SDR loss kernel for Trainium2 (8 NeuronCores).

Full inputs: pr_batch [32, 4, 131072] f32, t_batch [32, 4, 131072] f32.
Output: scalar f32 loss = -mean_b max_perm mean_j 10*log10((Td_j+eps)/(e_dot+eps)).

Sharding: pure data parallel on batch — 4 examples per core. Each core
computes, per example, the 24 length-T dot products (Pd_i, Td_j, C_ij),
then the tiny permutation/log/max tail on-device, outputting one value
per example. Host gathers 32 values, scales, means, negates.

Perf design (memory-bound target; HBM read ~16.8 MB/core is the floor):
- SWDGE DMA casts f32->bf16 during the HBM->SBUF load.
- Cross products pr_i*t_j: DVE tensor_tensor mult in bf16 (2x_1P mode,
  ~620ns per [128,1024]); the reduction of each product runs on the
  otherwise-idle TensorEngine as a ones-stationary matmul whose output
  AP folds the 512-column stream onto 16 PSUM columns (PSUM has_written
  accumulate; verified exact). One DVE reduce per example collapses the
  [128, 16 pairs x 16] PSUM block to [128, 16] per-partition partials.
- Self dots: ACT Square activation with free-dim accumulator.
- Partition reduction of accumulator columns: ones-matmul per example.
- Tail (selector matmul, ln, reduces) is exactly equivalent to:
    best[e] = sum_j ln(Td_j+eps) - min_p sum_j ln(e_dot[p,j]+eps)
  Host: loss = -mean(best * 10/(4 ln10)).
"""

import itertools
import os

import numpy as np

import concourse.bacc as bacc
import concourse.bass as bass
import concourse.mybir as mybir
import concourse.tile as tile
from concourse.bass_utils import run_bass_kernel_spmd

N_CORES = 8
B, NSRC, T = 32, 4, 131072
EB = B // N_CORES  # examples per core
P = 128
F = T // P  # 1024 free elems per partition per source
EPS = 1e-9
NPERM = 24
NPAIR = NSRC * NSRC  # 16 cross pairs
FOLD = 16  # PSUM columns per pair (stream folds 512 -> 16; >=16 avoids RAW)
NE = NPERM * NSRC + NSRC  # 100 output rows of the selector matmul
NVPAD = 64  # v_sb partitions: 0..7 = Pd/Td, 32..47 = C
SNR_SCALE = 10.0 / (NSRC * np.log(10.0))

AF = mybir.ActivationFunctionType
ALU = mybir.AluOpType
FP32 = mybir.dt.float32
BF16 = mybir.dt.bfloat16


def _selector_matrix() -> np.ndarray:
    """M [NE, NVPAD]: v rows 0..3 Pd_i, 4..7 Td_j, 32+i*4+j C[i,j].
    e_dot[(p,j)] = Pd[perm[p][j]] - 2*C[perm[p][j], j] + Td[j]; rows 96..99
    emit Td_j for the numerator."""
    perms = list(itertools.permutations(range(NSRC)))
    M = np.zeros((NE, NVPAD), dtype=np.float32)
    for pi, perm in enumerate(perms):
        for j in range(NSRC):
            m = pi * NSRC + j
            M[m, perm[j]] += 1.0
            M[m, 4 + j] += 1.0
            # C arrives replicated on all 128 partitions (PE colsum), so the
            # partition-sum in stage-A over-counts by 128x — fold 1/128 here.
            M[m, 32 + perm[j] * NSRC + j] += -2.0 / 128.0
    for j in range(NSRC):
        M[NPERM * NSRC + j, 4 + j] = 1.0
    return M


HF = F // 2  # half-chunk free size (512)


def _fold_ap(psum_tile, pair: int):
    """Matmul out AP: stream column n -> psum column pair*FOLD + (n % FOLD)."""
    sl = psum_tile[:, pair * FOLD : (pair + 1) * FOLD]
    return bass.AP(
        tensor=sl.tensor,
        offset=sl.offset,
        ap=[list(sl.ap[0]), [0, HF // FOLD], [1, FOLD]],
    )


def _build_bass() -> bass.Bass:
    # Bacc (not plain Bass): generate_event_semaphores splits sync waits to
    # <=1 per instruction, which this walrus build requires.
    nc = bacc.Bacc("TRN2")
    pr_h = nc.dram_tensor("pr", [EB, NSRC, T], FP32, kind="ExternalInput")
    t_h = nc.dram_tensor("t", [EB, NSRC, T], FP32, kind="ExternalInput")
    out_h = nc.dram_tensor("best", [EB, 1], FP32, kind="ExternalOutput")

    m2t_h = nc.inline_tensor(np.ascontiguousarray(_selector_matrix().T))  # [NVPAD, NE]
    ones_h = nc.inline_tensor(np.ones((P, 1), np.float32))
    onesm_h = nc.inline_tensor(np.ones((P, P), np.float32))

    with tile.TileContext(nc) as tc:
        with (
            tc.tile_pool(name="data", bufs=3) as data,
            tc.tile_pool(name="prod", bufs=6) as prod,
            tc.tile_pool(name="scratch", bufs=2) as scratch,
            tc.tile_pool(name="accum", bufs=2) as accum,
            tc.tile_pool(name="small", bufs=1) as small,
            tc.tile_pool(name="psum", bufs=1, space="PSUM") as psum,
            tc.tile_pool(name="psumc", bufs=2, space="PSUM") as psumc,
        ):
            ones_sb = small.tile([P, 1], FP32)
            nc.sync.dma_start(out=ones_sb, in_=ones_h[:, :])
            onesm_f = small.tile([P, P], FP32)
            nc.sync.dma_start(out=onesm_f, in_=onesm_h[:, :])
            onesm = small.tile([P, P], BF16)
            nc.vector.tensor_copy(onesm, onesm_f)
            m2t_sb = small.tile([NVPAD, NE], FP32)
            nc.sync.dma_start(out=m2t_sb, in_=m2t_h[:, :])

            # Early dummy Ln so the act-table pass picks a set containing
            # both Ln and Square (natural_log) and loads it once, off the
            # critical path.
            eps_sb = small.tile([EB, 1], FP32)
            nc.vector.memset(eps_sb, float(EPS))
            dummy_ln = small.tile([1, 1], FP32)
            nc.scalar.activation(
                out=dummy_ln, in_=eps_sb[0:1, :], func=AF.Ln, bias=eps_sb[0:1, :]
            )

            psum_va = psum.tile([8, EB], FP32, tag="va")
            psum_vd = psum.tile([NPAIR, EB], FP32, tag="vd")

            for e in range(EB):
                pr_t = data.tile([P, NSRC, F], BF16, tag="pr")
                t_t = data.tile([P, NSRC, F], BF16, tag="t")
                # SWDGE DMA with f32 -> bf16 cast during transfer, split in
                # half-T chunks so DVE can start after ~2MB instead of ~4MB.
                src_pr = pr_h[e].rearrange("s (p f) -> p s f", p=P)
                src_t = t_h[e].rearrange("s (p f) -> p s f", p=P)
                for h in range(2):
                    fs = slice(h * HF, (h + 1) * HF)
                    nc.gpsimd.dma_start(out=pr_t[:, :, fs], in_=src_pr[:, :, fs])
                    nc.gpsimd.dma_start(out=t_t[:, :, fs], in_=src_t[:, :, fs])

                a_act = accum.tile([P, 8], FP32, tag="a_act")
                a_dve = accum.tile([P, NPAIR], FP32, tag="a_dve")

                # Cross products on DVE (bf16 2x), reduced on PE via folded
                # ones-matmuls into psum_c, then one DVE reduce -> a_dve.
                psum_c = psumc.tile([P, NPAIR * FOLD], FP32, tag="pc")
                for h in range(2):
                    fs = slice(h * HF, (h + 1) * HF)
                    for i in range(NSRC):
                        for j in range(NSRC):
                            k = i * NSRC + j
                            z = prod.tile([P, HF], BF16, tag="z")
                            nc.vector.tensor_mul(
                                z[:, :], pr_t[:, i, fs], t_t[:, j, fs]
                            )
                            nc.tensor.matmul(
                                _fold_ap(psum_c, k),
                                lhsT=onesm,
                                rhs=z[:, :],
                                start=(h == 0),
                                stop=(h == 1),
                            )
                    if h == 1:
                        # Self dots on ACT (full-size; both halves present).
                        for i in range(NSRC):
                            sc_a = scratch.tile([P, F], BF16, tag="sa")
                            nc.scalar.activation(
                                out=sc_a,
                                in_=pr_t[:, i, :],
                                func=AF.Square,
                                accum_out=a_act[:, i : i + 1],
                            )
                        for j in range(NSRC):
                            sc_a = scratch.tile([P, F], BF16, tag="sa")
                            nc.scalar.activation(
                                out=sc_a,
                                in_=t_t[:, j, :],
                                func=AF.Square,
                                accum_out=a_act[:, 4 + j : 5 + j],
                            )
                nc.vector.tensor_reduce(
                    out=a_dve,
                    in_=psum_c.rearrange("p (k f) -> p k f", f=FOLD),
                    axis=mybir.AxisListType.X,
                    op=ALU.add,
                )
                # Partition-reduce accumulator columns via ones-matmul.
                nc.tensor.matmul(
                    psum_va[:, e : e + 1], lhsT=a_act, rhs=ones_sb, start=True, stop=True
                )
                nc.tensor.matmul(
                    psum_vd[:, e : e + 1], lhsT=a_dve, rhs=ones_sb, start=True, stop=True
                )

            # Tail: tiny. v_sb [NVPAD, EB] -> selector matmul -> ln -> reduces.
            v_sb = small.tile([NVPAD, EB], FP32)
            nc.vector.memset(v_sb, 0.0)
            nc.scalar.copy(out=v_sb[0:8, :], in_=psum_va[:, :])
            nc.scalar.copy(out=v_sb[32 : 32 + NPAIR, :], in_=psum_vd[:, :])

            psum_e = psum.tile([EB, NE], FP32, tag="ed")
            nc.tensor.matmul(psum_e, lhsT=v_sb, rhs=m2t_sb, start=True, stop=True)

            ln_sb = small.tile([EB, NE], FP32)
            nc.scalar.activation(
                out=ln_sb, in_=psum_e, func=AF.Ln, bias=eps_sb[:, :], scale=1.0
            )
            s_perm = small.tile([EB, NPERM], FP32)
            nc.vector.tensor_reduce(
                out=s_perm,
                in_=ln_sb[:, 0 : NPERM * NSRC].rearrange("p (q j) -> p q j", j=NSRC),
                axis=mybir.AxisListType.X,
                op=ALU.add,
            )
            tsum = small.tile([EB, 1], FP32)
            nc.vector.tensor_reduce(
                out=tsum,
                in_=ln_sb[:, NPERM * NSRC : NE],
                axis=mybir.AxisListType.X,
                op=ALU.add,
            )
            mmin = small.tile([EB, 1], FP32)
            nc.vector.tensor_reduce(
                out=mmin, in_=s_perm, axis=mybir.AxisListType.X, op=ALU.min
            )
            diff = small.tile([EB, 1], FP32)
            nc.vector.tensor_sub(diff, tsum, mmin)
            nc.sync.dma_start(out=out_h[:, :], in_=diff)

    nc.finalize()
    return nc


_NC_CACHE = None


def _get_nc() -> bass.Bass:
    global _NC_CACHE
    if _NC_CACHE is None:
        _NC_CACHE = _build_bass()
    return _NC_CACHE


def run_device(pr_batch: np.ndarray, t_batch: np.ndarray, **run_kwargs):
    pr = np.ascontiguousarray(pr_batch, dtype=np.float32)
    t = np.ascontiguousarray(t_batch, dtype=np.float32)
    assert pr.shape == (B, NSRC, T) and t.shape == (B, NSRC, T)
    nc = _get_nc()
    in_maps = [
        {"pr": pr[c * EB : (c + 1) * EB], "t": t[c * EB : (c + 1) * EB]}
        for c in range(N_CORES)
    ]
    return run_bass_kernel_spmd(nc, in_maps, core_ids=list(range(N_CORES)), **run_kwargs)


def kernel(pr_batch: np.ndarray, t_batch: np.ndarray) -> np.ndarray:
    res = run_device(pr_batch, t_batch)
    best = np.concatenate([r["best"].reshape(-1) for r in res.results])
    loss = -np.float32(SNR_SCALE) * best.astype(np.float32).mean(dtype=np.float32)
    return np.float32(loss)


if __name__ == "__main__":
    rng = np.random.default_rng(0)
    pr = rng.standard_normal((B, NSRC, T), dtype=np.float32)
    t = rng.standard_normal((B, NSRC, T), dtype=np.float32)
    print(kernel(pr, t))
